# revision 8
# baseline (speedup 1.0000x reference)
"""GQA attention (dense_transformer) on 8 TRN2 NeuronCores.

Sharding: tensor-parallel over heads. Core c computes q-heads {2c, 2c+1}
(their shared kv head is c//2): column-parallel Wq/Wk/Wv, row-parallel Wo;
the 8 partial o_proj outputs are summed on the host.

v3 design (vs the f32r baseline):
  - all matmul operands bf16 (fp8 propagates ~3% element error straight
    to the output through the random-sign dot products here; bf16 keeps
    the stack at ~0.5%). PSUM accumulation stays fp32.
  - exp emitted 1024-wide ([sk-pair, sq] PSUM groups) straight to bf16.
  - RoPE applied in place (q_st/kv_st double as the roped tensors).
  - X^T streamed per 512-seq window (triple buffered), weights resident.
  - phase interleave: proj(b1) passes fill PE slack inside B(b0,*)'s
    sqg loop; C(b0) fills B(b1,*); only C(b1) trails.
  - PSUM budgeted <=8 banks in every region (2-bank proj passes).
"""

import math

import ml_dtypes
import numpy as np

import concourse.bacc as bacc_mod
import concourse.mybir as mybir
import concourse.tile as tile
from concourse.bass_utils import run_bass_kernel_spmd

HIDDEN = 2048
N_HEADS = 16
N_KV_HEADS = 4
HEAD_DIM = 128
ROPE_THETA = 10000.0
B = 2
S = 2048
N_CORES = 8
NH_LOC = N_HEADS // N_CORES  # 2 q heads per core
P = 128
F32 = mybir.dt.float32
BF16 = mybir.dt.bfloat16
SCALE = 1.0 / math.sqrt(HEAD_DIM)

KH = HIDDEN // P  # 16 contraction chunks
NW = B * 4  # 8 seq windows of 512
NSK = S // P  # 16 sk chunks


def _rope_tables(s, d, theta):
    inv_freq = 1.0 / (theta ** (np.arange(0, d, 2, dtype=np.float32) / d))
    t = np.arange(s, dtype=np.float32)
    freqs = np.outer(t, inv_freq).astype(np.float32)  # [S, d/2]
    emb = np.concatenate([freqs, freqs], axis=-1)  # [S, d]
    cos_t = np.ascontiguousarray(np.cos(emb).astype(np.float32).T)  # [d, S]
    sin_t = np.ascontiguousarray(np.sin(emb).astype(np.float32).T)
    return cos_t, sin_t


def _rot_matrix_t(d):
    # R @ q == rotate_half(q); stationary operand is R^T (matmul computes
    # lhsT.T @ rhs).
    r = np.zeros((d, d), dtype=np.float32)
    h = d // 2
    for i in range(h):
        r[i, i + h] = -1.0
        r[i + h, i] = 1.0
    return np.ascontiguousarray(r.T)


def _build(add_mask):
    nc = bacc_mod.Bacc()
    xt_d = nc.dram_tensor("xtb", [P, NW, KH, 512], BF16, kind="ExternalInput")
    wq_d = nc.dram_tensor("wqb", [P, KH, NH_LOC * P], BF16, kind="ExternalInput")
    wk_d = nc.dram_tensor("wkb", [P, KH, P], BF16, kind="ExternalInput")
    wv_d = nc.dram_tensor("wvb", [P, KH, P], BF16, kind="ExternalInput")
    wo_d = nc.dram_tensor("wob", [P, NH_LOC, HIDDEN], BF16, kind="ExternalInput")
    ones_d = nc.dram_tensor("onesb", [P, P], BF16, kind="ExternalInput")
    cosb_d = nc.dram_tensor("cosb", [P, S], BF16, kind="ExternalInput")
    sinf_d = nc.dram_tensor("sinf", [P, S], F32, kind="ExternalInput")
    rt_d = nc.dram_tensor("rt", [P, P], BF16, kind="ExternalInput")
    id_d = nc.dram_tensor("ident", [P, P], BF16, kind="ExternalInput")
    if add_mask:
        mt_d = nc.dram_tensor("mask_t", [S, S], F32, kind="ExternalInput")
    out_d = nc.dram_tensor("out", [B * S, HIDDEN], BF16, kind="ExternalOutput")

    with tile.TileContext(nc) as tc:
        with (
            tc.tile_pool(name="consts", bufs=1) as consts,
            tc.tile_pool(name="persist", bufs=1) as persist,
            tc.tile_pool(name="stage", bufs=1) as stage,
            tc.tile_pool(name="xstage", bufs=3) as xstage,
        ):
            # ---- persistent SBUF ----
            wq_sb = persist.tile([P, KH, NH_LOC * P], BF16, tag="wq")
            wk_sb = persist.tile([P, KH, P], BF16, tag="wk")
            wv_sb = persist.tile([P, KH, P], BF16, tag="wv")
            wo_sb = persist.tile([P, NH_LOC, HIDDEN], BF16, tag="wo")
            ones_sb = consts.tile([P, P], BF16, tag="ones")
            cos_sb = consts.tile([P, S], BF16, tag="cos")
            sin_sb = consts.tile([P, S], F32, tag="sin")
            rt_sb = consts.tile([P, P], BF16, tag="rt")
            id_sb = consts.tile([P, P], BF16, tag="id")

            # q_st/kv_st are roped in place; [:,0,:] of kv_st is k, [:,1,:] v
            q_st = [persist.tile([P, NH_LOC, S], BF16, tag=f"qst{bi}",
                                 name=f"qst{bi}") for bi in range(B)]
            kv_st = [persist.tile([P, 2, S], BF16, tag=f"kvst{bi}",
                                  name=f"kvst{bi}") for bi in range(B)]
            vn = [persist.tile([P, NSK, P], BF16, tag=f"vn{bi}",
                               name=f"vn{bi}") for bi in range(B)]
            outn = [persist.tile([P, NH_LOC, S], BF16, tag=f"on{bi}",
                                 name=f"on{bi}") for bi in range(B)]

            # ---- input DMAs: kv weights + first xt window lead so the
            # first (kv) projection pass starts as early as possible ----
            nc.sync.dma_start(out=wk_sb, in_=wk_d[:, :, :])
            nc.sync.dma_start(out=wv_sb, in_=wv_d[:, :, :])
            # consts on the scalar (ACT) HWDGE queue — off the critical path
            nc.scalar.dma_start(out=cos_sb, in_=cosb_d[:, :])
            nc.scalar.dma_start(out=sin_sb, in_=sinf_d[:, :])
            nc.scalar.dma_start(out=rt_sb, in_=rt_d[:, :])
            nc.scalar.dma_start(out=id_sb, in_=id_d[:, :])
            nc.scalar.dma_start(out=ones_sb, in_=ones_d[:, :])
            nc.scalar.dma_start(out=wo_sb, in_=wo_d[:, :, :])
            if add_mask:
                mask_sb = persist.tile([P, NSK, S], F32, tag="mask")
                nc.scalar.dma_start(
                    out=mask_sb, in_=mt_d.rearrange("(c p) m -> p c m", p=P)
                )
            # prewarm the exp table set during phase A
            warm = stage.tile([P, 8], BF16, tag="warm")
            nc.scalar.activation(
                warm, cos_sb[:, :8], mybir.ActivationFunctionType.Exp
            )

            # xt windows, streamed + triple buffered
            xw_tiles = {}

            def get_xw(w):
                if w not in xw_tiles:
                    t = xstage.tile([P, KH, 512], BF16, tag="xw", bufs=3,
                                    name=f"xw{w}")
                    nc.sync.dma_start(out=t, in_=xt_d[:, w])
                    xw_tiles[w] = t
                return xw_tiles[w]

            # first window in quarters so matmuls start after ~2 of 16 chunks
            xw0 = xstage.tile([P, KH, 512], BF16, tag="xw", bufs=3, name="xw0")
            for cq in range(4):
                nc.sync.dma_start(
                    out=xw0[:, 4 * cq : 4 * cq + 4, :],
                    in_=xt_d[:, 0, 4 * cq : 4 * cq + 4, :],
                )
            xw_tiles[0] = xw0
            nc.sync.dma_start(out=wq_sb, in_=wq_d[:, :, :])

            # ------------- emission helpers -------------
            def emit_proj_pass(pool, bi, w, which):
                """One 2-bank projection pass: 32 matmuls + 1 drain."""
                pp = pool.tile([P, 2, 512], F32, tag="pp",
                               name=f"pp{bi}{w}{which}")
                xw = get_xw(bi * 4 + w)
                for c in range(KH):
                    st_, sp_ = c == 0, c == KH - 1
                    if which == "q":
                        nc.tensor.matmul(
                            pp[:, 0, :], wq_sb[:, c, 0:P], xw[:, c, :],
                            start=st_, stop=sp_,
                        )
                        nc.tensor.matmul(
                            pp[:, 1, :], wq_sb[:, c, P : 2 * P], xw[:, c, :],
                            start=st_, stop=sp_,
                        )
                    else:
                        nc.tensor.matmul(
                            pp[:, 0, :], wk_sb[:, c, :], xw[:, c, :],
                            start=st_, stop=sp_,
                        )
                        nc.tensor.matmul(
                            pp[:, 1, :], wv_sb[:, c, :], xw[:, c, :],
                            start=st_, stop=sp_,
                        )
                dst = q_st[bi] if which == "q" else kv_st[bi]
                sl = slice(w * 512, (w + 1) * 512)
                nc.scalar.copy(dst[:, :, sl], pp)

            def emit_rot_chunk(bi, src_ap, ji, ch, pr_pool, tt_pool):
                sl = slice(ch * 512, (ch + 1) * 512)
                pr = pr_pool.tile([P, 512], F32, tag="pr", bufs=4,
                                  name=f"pr{bi}{ji}{ch}")
                nc.tensor.matmul(pr, rt_sb, src_ap[:, sl],
                                 start=True, stop=True)
                t_t = tt_pool.tile([P, 512], BF16, tag="tt", bufs=4,
                                   name=f"tt{bi}{ji}{ch}")
                nc.vector.tensor_mul(t_t, pr, sin_sb[:, sl])
                x_t = tt_pool.tile([P, 512], BF16, tag="xt2", bufs=4,
                                   name=f"xt2{bi}{ji}{ch}")
                nc.vector.tensor_mul(x_t, src_ap[:, sl], cos_sb[:, sl])
                nc.vector.tensor_add(src_ap[:, sl], x_t, t_t)

            def emit_vt_group(bi, g4, pv_pool):
                pv = pv_pool.tile([P, 512], BF16, tag="pv", bufs=2,
                                  name=f"pv{bi}{g4}")
                for j in range(4):
                    blk = g4 * 4 + j
                    nc.tensor.matmul(
                        pv[:, j * P : (j + 1) * P],
                        kv_st[bi][:, 1, blk * P : (blk + 1) * P],
                        id_sb, is_transpose=True, start=True, stop=True,
                    )
                nc.scalar.copy(vn[bi][:, g4 * 4 : g4 * 4 + 4, :], pv)

            def emit_rot_vt(bi, pr_pool, tt_pool):
                """In-place RoPE + V transpose. k/q0 chunks interleaved
                (B consumes k and q0 first); q1 chunks alternate with vT
                groups as PE filler while the DVE chain catches up."""
                k_ap = kv_st[bi][:, 0, :]
                for ch in range(4):
                    emit_rot_chunk(bi, k_ap, 0, ch, pr_pool, tt_pool)
                    emit_rot_chunk(bi, q_st[bi][:, 0, :], 1, ch,
                                   pr_pool, tt_pool)
                for ch in range(4):
                    emit_rot_chunk(bi, q_st[bi][:, 1, :], 2, ch,
                                   pr_pool, tt_pool)
                    emit_vt_group(bi, ch, pr_pool)

            def emit_b_unit(bi, m, pools, fillers=None, micro=None,
                            micro_rate=2, sqg_hook=None):
                """One (batch, head) attention unit: 4 sqg of 8 sk-pairs.
                micro: queue of small PE tasks drained micro_rate per sk-pair
                (fine-grained interleave); fillers: one big task per sqg."""
                psc, pout, psum2, expool, recpool = pools
                for sqg in range(4):
                    qsl = slice(sqg * 512, (sqg + 1) * 512)
                    out_ps = pout.tile([P, 512], F32, tag="out",
                                       name=f"out{bi}{m}{sqg}")
                    sum_ps = psum2.tile([P, 512], F32, tag="sum",
                                        name=f"sum{bi}{m}{sqg}")
                    for g in range(NSK // 2):
                        sc2 = psc.tile([P, 2, 512], F32, tag="sc",
                                       name=f"sc{bi}{m}{sqg}{g}")
                        for j in range(2):
                            sk = 2 * g + j
                            nc.tensor.matmul(
                                sc2[:, j, :],
                                kv_st[bi][:, 0, sk * P : (sk + 1) * P],
                                q_st[bi][:, m, qsl],
                                start=True, stop=True,
                            )
                        if add_mask:
                            for j in range(2):
                                nc.vector.scalar_tensor_tensor(
                                    sc2[:, j, :], sc2[:, j, :], SCALE,
                                    mask_sb[:, 2 * g + j, qsl],
                                    op0=mybir.AluOpType.mult,
                                    op1=mybir.AluOpType.add,
                                )
                        ex2 = expool.tile([P, 2, 512], BF16, tag="ex", bufs=3,
                                          name=f"ex{bi}{m}{sqg}{g}")
                        if add_mask:
                            nc.scalar.activation(
                                ex2, sc2, mybir.ActivationFunctionType.Exp,
                            )
                        else:
                            nc.scalar.activation(
                                ex2, sc2, mybir.ActivationFunctionType.Exp,
                                scale=SCALE,
                            )
                        for j in range(2):
                            sk = 2 * g + j
                            st_, sp_ = sk == 0, sk == NSK - 1
                            nc.tensor.matmul(
                                out_ps, vn[bi][:, sk, :], ex2[:, j, :],
                                start=st_, stop=sp_,
                            )
                            nc.tensor.matmul(
                                sum_ps, ones_sb, ex2[:, j, :],
                                start=st_, stop=sp_,
                            )
                        if micro:
                            for _ in range(micro_rate):
                                if micro:
                                    micro.pop(0)()
                    rec = recpool.tile([P, 512], F32, tag="rec", bufs=2,
                                       name=f"rec{bi}{m}{sqg}")
                    nc.vector.reciprocal_approx_fast(rec, sum_ps)
                    nc.vector.tensor_mul(outn[bi][:, m, qsl], out_ps, rec)
                    if fillers:
                        fillers.pop(0)()
                    if sqg_hook is not None:
                        sqg_hook(sqg)

            c_ob = {}

            def c_task(bi, sqt, hc, po_pool, ob_pool, drain_eng="vector"):
                """One o_proj micro-task: [128,512] psum, 2 MMs, 1 drain;
                hc==3 also DMAs the assembled [128,2048] row block out."""
                ssl = slice(sqt * P, (sqt + 1) * P)
                if hc == 0:
                    c_ob[(bi, sqt)] = ob_pool.tile(
                        [P, HIDDEN], BF16, tag="ob", bufs=3,
                        name=f"ob{bi}{sqt}")
                ob = c_ob[(bi, sqt)]
                po = po_pool.tile([P, 512], F32, tag="po", bufs=2,
                                  name=f"po{bi}{sqt}{hc}")
                col = hc * 512
                for dc in range(NH_LOC):
                    nc.tensor.matmul(
                        po,
                        outn[bi][:, dc, ssl],
                        wo_sb[:, dc, col : col + 512],
                        start=dc == 0, stop=dc == NH_LOC - 1,
                    )
                osl = slice(col, col + 512)
                if drain_eng == "vector":
                    nc.vector.tensor_copy(ob[:, osl], po)
                else:
                    nc.scalar.copy(ob[:, osl], po)
                if hc == 3:
                    nc.sync.dma_start(
                        out=out_d[bi * S + sqt * P : bi * S + (sqt + 1) * P, :],
                        in_=ob,
                    )

            # ------------- the program -------------
            # A(b0): dense projection passes, DMA-paced
            psA_cm = tc.tile_pool(name="psA", bufs=3, space="PSUM")
            psA = psA_cm.__enter__()
            for w in range(4):
                emit_proj_pass(psA, 0, w, "kv")
                emit_proj_pass(psA, 0, w, "q")
            psA_cm.__exit__(None, None, None)

            # rot + vT for b0
            rv_cm = tc.tile_pool(name="rv0", bufs=2, space="PSUM")
            rv = rv_cm.__enter__()
            emit_rot_vt(0, rv, stage)
            rv_cm.__exit__(None, None, None)

            # B(b0,*) with proj(b1) passes as fillers
            fill_b1 = []
            psF_cm = tc.tile_pool(name="psF", bufs=1, space="PSUM")
            psF = psF_cm.__enter__()
            for w in range(4):
                for which in ("kv", "q"):
                    fill_b1.append(
                        lambda w=w, wh=which: emit_proj_pass(psF, 1, w, wh)
                    )

            for m in range(NH_LOC):
                pools_cm = [
                    tc.tile_pool(name=f"psc0{m}", bufs=2, space="PSUM"),
                    tc.tile_pool(name=f"pout0{m}", bufs=1, space="PSUM"),
                    tc.tile_pool(name=f"psum0{m}", bufs=1, space="PSUM"),
                    tc.tile_pool(name=f"ex0{m}", bufs=3),
                    tc.tile_pool(name=f"rec0{m}", bufs=2),
                ]
                pools = [p.__enter__() for p in pools_cm]
                emit_b_unit(0, m, pools, fill_b1)
                for p in reversed(pools_cm):
                    p.__exit__(None, None, None)
            psF_cm.__exit__(None, None, None)

            # rot + vT for b1
            rv1_cm = tc.tile_pool(name="rv1", bufs=2, space="PSUM")
            rv1 = rv1_cm.__enter__()
            emit_rot_vt(1, rv1, stage)
            rv1_cm.__exit__(None, None, None)

            # B(b1,*): h0 interleaves C(b0) micro-tasks, h1 C(b1)'s
            poF_cm = tc.tile_pool(name="poF", bufs=2, space="PSUM")
            poF = poF_cm.__enter__()
            obF_cm = tc.tile_pool(name="obF", bufs=3)
            obF = obF_cm.__enter__()

            micro_c0 = [
                (lambda sqt=sqt, hc=hc: c_task(0, sqt, hc, poF, obF))
                for sqt in range(16) for hc in range(4)
            ]
            micro_c1 = []

            def c1_hook(sqg):
                # outn[1][:, 1, window sqg] just completed -> its sqt rows
                micro_c1.extend(
                    (lambda sqt=sqt, hc=hc: c_task(1, sqt, hc, poF, obF))
                    for sqt in range(4 * sqg, 4 * sqg + 4) for hc in range(4)
                )

            for m in range(NH_LOC):
                pools_cm = [
                    tc.tile_pool(name=f"psc1{m}", bufs=2, space="PSUM"),
                    tc.tile_pool(name=f"pout1{m}", bufs=1, space="PSUM"),
                    tc.tile_pool(name=f"psum1{m}", bufs=1, space="PSUM"),
                    tc.tile_pool(name=f"ex1{m}", bufs=3),
                    tc.tile_pool(name=f"rec1{m}", bufs=2),
                ]
                pools = [p.__enter__() for p in pools_cm]
                if m == 0:
                    emit_b_unit(1, m, pools, micro=micro_c0, micro_rate=2)
                else:
                    emit_b_unit(1, m, pools, micro=micro_c1, micro_rate=3,
                                sqg_hook=c1_hook)
                for p in reversed(pools_cm):
                    p.__exit__(None, None, None)

            # dense tail: whatever C(b1) tasks remain (last window)
            for t in micro_c1:
                t()
            obF_cm.__exit__(None, None, None)
            poF_cm.__exit__(None, None, None)

    nc.compile()
    return nc


_BUILD_CACHE = {}
LAST_RESULT = None


def _get_nc(add_mask):
    if add_mask not in _BUILD_CACHE:
        _BUILD_CACHE[add_mask] = _build(add_mask)
    return _BUILD_CACHE[add_mask]


def kernel(hidden_states, attention_mask, Wq, Wk, Wv, Wo):
    hidden_states = np.asarray(hidden_states, dtype=np.float32)
    attention_mask = np.asarray(attention_mask, dtype=np.float32)
    Wq = np.asarray(Wq, dtype=np.float32)
    Wk = np.asarray(Wk, dtype=np.float32)
    Wv = np.asarray(Wv, dtype=np.float32)
    Wo = np.asarray(Wo, dtype=np.float32)

    b, s, hidden = hidden_states.shape
    assert (b, s, hidden) == (B, S, HIDDEN)

    add_mask = bool(np.any(attention_mask))
    nc = _get_nc(add_mask)

    bf16 = ml_dtypes.bfloat16

    # X^T packed [p, w, c, s512]: hidden = c*128+p, seq-global = w*512+s
    xt = hidden_states.reshape(b * s, hidden).T  # [2048, 4096]
    xtb = np.ascontiguousarray(
        xt.reshape(KH, P, NW, 512).transpose(1, 2, 0, 3)
    ).astype(bf16)

    cos_t, sin_t = _rope_tables(s, HEAD_DIM, ROPE_THETA)
    cosb = cos_t.astype(bf16)
    rt = _rot_matrix_t(P).astype(bf16)
    ident = np.eye(P, dtype=np.float32).astype(bf16)
    onesb = np.ones((P, P), dtype=np.float32).astype(bf16)

    in_maps = []
    for c in range(N_CORES):
        kv = c // 2
        wq_c = Wq[:, c * NH_LOC * HEAD_DIM : (c + 1) * NH_LOC * HEAD_DIM]
        wk_c = Wk[:, kv * HEAD_DIM : (kv + 1) * HEAD_DIM]
        wv_c = Wv[:, kv * HEAD_DIM : (kv + 1) * HEAD_DIM]
        wo_c = Wo[c * NH_LOC * HEAD_DIM : (c + 1) * NH_LOC * HEAD_DIM, :]
        im = {
            "xtb": xtb,
            "wqb": np.ascontiguousarray(
                wq_c.reshape(KH, P, NH_LOC * P).transpose(1, 0, 2)
            ).astype(bf16),
            "wkb": np.ascontiguousarray(
                wk_c.reshape(KH, P, P).transpose(1, 0, 2)
            ).astype(bf16),
            "wvb": np.ascontiguousarray(
                wv_c.reshape(KH, P, P).transpose(1, 0, 2)
            ).astype(bf16),
            "wob": np.ascontiguousarray(
                wo_c.reshape(NH_LOC, P, HIDDEN).transpose(1, 0, 2)
            ).astype(bf16),
            "onesb": onesb,
            "cosb": cosb,
            "sinf": sin_t,
            "rt": rt,
            "ident": ident,
        }
        if add_mask:
            im["mask_t"] = np.ascontiguousarray(attention_mask[0, 0].T)
        in_maps.append(im)

    res = run_bass_kernel_spmd(nc, in_maps, core_ids=list(range(N_CORES)))
    global LAST_RESULT
    LAST_RESULT = res
    out = np.zeros((b * s, hidden), dtype=np.float32)
    for r in res.results:
        out += np.asarray(r["out"], dtype=np.float32)
    return out.reshape(b, s, hidden)


# revision 12
# speedup vs baseline: 1.3424x; 1.3424x over previous
"""GQA attention (dense_transformer) on 8 TRN2 NeuronCores.

Sharding: tensor-parallel over heads. Core c computes q-heads {2c, 2c+1}
(their shared kv head is c//2): column-parallel Wq/Wk/Wv, row-parallel Wo;
the 8 partial o_proj outputs are summed on the host.

v3 design (vs the f32r baseline):
  - all matmul operands bf16 (fp8 propagates ~3% element error straight
    to the output through the random-sign dot products here; bf16 keeps
    the stack at ~0.5%). PSUM accumulation stays fp32.
  - exp emitted 1024-wide ([sk-pair, sq] PSUM groups) straight to bf16.
  - RoPE applied in place (q_st/kv_st double as the roped tensors).
  - X^T streamed per 512-seq window (triple buffered), weights resident.
  - phase interleave: proj(b1) passes fill PE slack inside B(b0,*)'s
    sqg loop; C(b0) fills B(b1,*); only C(b1) trails.
  - PSUM budgeted <=8 banks in every region (2-bank proj passes).
"""

import math

import ml_dtypes
import numpy as np

import concourse.bacc as bacc_mod
import concourse.mybir as mybir
import concourse.tile as tile
from concourse.bass_utils import run_bass_kernel_spmd

HIDDEN = 2048
N_HEADS = 16
N_KV_HEADS = 4
HEAD_DIM = 128
ROPE_THETA = 10000.0
B = 2
S = 2048
N_CORES = 8
NH_LOC = N_HEADS // N_CORES  # 2 q heads per core
P = 128
F32 = mybir.dt.float32
BF16 = mybir.dt.bfloat16
SCALE = 1.0 / math.sqrt(HEAD_DIM)

KH = HIDDEN // P  # 16 contraction chunks
NW = B * 4  # 8 seq windows of 512
NSK = S // P  # 16 sk chunks


def _rope_tables(s, d, theta):
    inv_freq = 1.0 / (theta ** (np.arange(0, d, 2, dtype=np.float32) / d))
    t = np.arange(s, dtype=np.float32)
    freqs = np.outer(t, inv_freq).astype(np.float32)  # [S, d/2]
    emb = np.concatenate([freqs, freqs], axis=-1)  # [S, d]
    cos_t = np.ascontiguousarray(np.cos(emb).astype(np.float32).T)  # [d, S]
    sin_t = np.ascontiguousarray(np.sin(emb).astype(np.float32).T)
    return cos_t, sin_t


def _rot_matrix_t(d):
    # R @ q == rotate_half(q); stationary operand is R^T (matmul computes
    # lhsT.T @ rhs).
    r = np.zeros((d, d), dtype=np.float32)
    h = d // 2
    for i in range(h):
        r[i, i + h] = -1.0
        r[i + h, i] = 1.0
    return np.ascontiguousarray(r.T)


def _build(add_mask):
    nc = bacc_mod.Bacc()
    xt_d = nc.dram_tensor("xtb", [P, NW, KH, 512], BF16, kind="ExternalInput")
    wq_d = nc.dram_tensor("wqb", [P, KH, NH_LOC * P], BF16, kind="ExternalInput")
    wk_d = nc.dram_tensor("wkb", [P, KH, P], BF16, kind="ExternalInput")
    wv_d = nc.dram_tensor("wvb", [P, KH, P], BF16, kind="ExternalInput")
    wo_d = nc.dram_tensor("wob", [P, NH_LOC, HIDDEN], BF16, kind="ExternalInput")
    ones_d = nc.dram_tensor("onesb", [P, P], BF16, kind="ExternalInput")
    cosb_d = nc.dram_tensor("cosb", [P, S], BF16, kind="ExternalInput")
    sinf_d = nc.dram_tensor("sinf", [P, S], F32, kind="ExternalInput")
    rt_d = nc.dram_tensor("rt", [P, P], BF16, kind="ExternalInput")
    id_d = nc.dram_tensor("ident", [P, P], BF16, kind="ExternalInput")
    if add_mask:
        mt_d = nc.dram_tensor("mask_t", [S, S], F32, kind="ExternalInput")
    out_d = nc.dram_tensor("out", [B * S, HIDDEN], BF16, kind="ExternalOutput")

    with tile.TileContext(nc) as tc:
        with (
            tc.tile_pool(name="consts", bufs=1) as consts,
            tc.tile_pool(name="persist", bufs=1) as persist,
            tc.tile_pool(name="stage", bufs=1) as stage,
            tc.tile_pool(name="xstage", bufs=3) as xstage,
        ):
            # ---- persistent SBUF ----
            wq_sb = persist.tile([P, KH, NH_LOC * P], BF16, tag="wq")
            wk_sb = persist.tile([P, KH, P], BF16, tag="wk")
            wv_sb = persist.tile([P, KH, P], BF16, tag="wv")
            wo_sb = persist.tile([P, NH_LOC, HIDDEN], BF16, tag="wo")
            ones_sb = consts.tile([P, P], BF16, tag="ones")
            cos_sb = consts.tile([P, S], BF16, tag="cos")
            sin_sb = consts.tile([P, S], F32, tag="sin")
            rt_sb = consts.tile([P, P], BF16, tag="rt")
            id_sb = consts.tile([P, P], BF16, tag="id")

            # q_st/kv_st are roped in place; [:,0,:] of kv_st is k, [:,1,:] v
            q_st = [persist.tile([P, NH_LOC, S], BF16, tag=f"qst{bi}",
                                 name=f"qst{bi}") for bi in range(B)]
            kv_st = [persist.tile([P, 2, S], BF16, tag=f"kvst{bi}",
                                  name=f"kvst{bi}") for bi in range(B)]
            vn = [persist.tile([P, NSK, P], BF16, tag=f"vn{bi}",
                               name=f"vn{bi}") for bi in range(B)]
            outn = [persist.tile([P, NH_LOC, S], BF16, tag=f"on{bi}",
                                 name=f"on{bi}") for bi in range(B)]

            # ---- input DMAs: kv weights + first xt window lead so the
            # first (kv) projection pass starts as early as possible ----
            nc.sync.dma_start(out=wk_sb, in_=wk_d[:, :, :])
            nc.sync.dma_start(out=wv_sb, in_=wv_d[:, :, :])
            # consts on the scalar (ACT) HWDGE queue — off the critical path
            nc.scalar.dma_start(out=cos_sb, in_=cosb_d[:, :])
            nc.scalar.dma_start(out=sin_sb, in_=sinf_d[:, :])
            nc.scalar.dma_start(out=rt_sb, in_=rt_d[:, :])
            nc.scalar.dma_start(out=id_sb, in_=id_d[:, :])
            nc.scalar.dma_start(out=ones_sb, in_=ones_d[:, :])
            nc.scalar.dma_start(out=wo_sb, in_=wo_d[:, :, :])
            if add_mask:
                mask_sb = persist.tile([P, NSK, S], F32, tag="mask")
                nc.scalar.dma_start(
                    out=mask_sb, in_=mt_d.rearrange("(c p) m -> p c m", p=P)
                )
            # prewarm the exp table set during phase A
            warm = stage.tile([P, 8], BF16, tag="warm")
            nc.scalar.activation(
                warm, cos_sb[:, :8], mybir.ActivationFunctionType.Exp
            )

            # xt windows, streamed + triple buffered
            xw_tiles = {}

            def get_xw(w):
                if w not in xw_tiles:
                    t = xstage.tile([P, KH, 512], BF16, tag="xw", bufs=3,
                                    name=f"xw{w}")
                    nc.sync.dma_start(out=t, in_=xt_d[:, w])
                    xw_tiles[w] = t
                return xw_tiles[w]

            # first window in quarters so matmuls start after ~2 of 16 chunks
            xw0 = xstage.tile([P, KH, 512], BF16, tag="xw", bufs=3, name="xw0")
            for cq in range(4):
                nc.sync.dma_start(
                    out=xw0[:, 4 * cq : 4 * cq + 4, :],
                    in_=xt_d[:, 0, 4 * cq : 4 * cq + 4, :],
                )
            xw_tiles[0] = xw0
            nc.sync.dma_start(out=wq_sb, in_=wq_d[:, :, :])

            # ------------- emission helpers -------------
            def emit_proj_pass(pool, bi, w, which):
                """One 2-bank projection pass: 32 matmuls + 1 drain."""
                pp = pool.tile([P, 2, 512], F32, tag="pp",
                               name=f"pp{bi}{w}{which}")
                xw = get_xw(bi * 4 + w)
                for c in range(KH):
                    st_, sp_ = c == 0, c == KH - 1
                    if which == "q":
                        nc.tensor.matmul(
                            pp[:, 0, :], wq_sb[:, c, 0:P], xw[:, c, :],
                            start=st_, stop=sp_,
                        )
                        nc.tensor.matmul(
                            pp[:, 1, :], wq_sb[:, c, P : 2 * P], xw[:, c, :],
                            start=st_, stop=sp_,
                        )
                    else:
                        nc.tensor.matmul(
                            pp[:, 0, :], wk_sb[:, c, :], xw[:, c, :],
                            start=st_, stop=sp_,
                        )
                        nc.tensor.matmul(
                            pp[:, 1, :], wv_sb[:, c, :], xw[:, c, :],
                            start=st_, stop=sp_,
                        )
                dst = q_st[bi] if which == "q" else kv_st[bi]
                sl = slice(w * 512, (w + 1) * 512)
                nc.scalar.copy(dst[:, :, sl], pp)

            def emit_rot_chunk(bi, src_ap, ji, ch, pr_pool, tt_pool):
                sl = slice(ch * 512, (ch + 1) * 512)
                pr = pr_pool.tile([P, 512], F32, tag="pr", bufs=2,
                                  name=f"pr{bi}{ji}{ch}")
                nc.tensor.matmul(pr, rt_sb, src_ap[:, sl],
                                 start=True, stop=True)
                t_t = tt_pool.tile([P, 512], BF16, tag="tt", bufs=4,
                                   name=f"tt{bi}{ji}{ch}")
                nc.vector.tensor_mul(t_t, pr, sin_sb[:, sl])
                x_t = tt_pool.tile([P, 512], BF16, tag="xt2", bufs=4,
                                   name=f"xt2{bi}{ji}{ch}")
                nc.vector.tensor_mul(x_t, src_ap[:, sl], cos_sb[:, sl])
                nc.vector.tensor_add(src_ap[:, sl], x_t, t_t)

            def emit_vt_group(bi, g4, pv_pool):
                pv = pv_pool.tile([P, 512], BF16, tag="pv", bufs=2,
                                  name=f"pv{bi}{g4}")
                for j in range(4):
                    blk = g4 * 4 + j
                    nc.tensor.matmul(
                        pv[:, j * P : (j + 1) * P],
                        kv_st[bi][:, 1, blk * P : (blk + 1) * P],
                        id_sb, is_transpose=True, start=True, stop=True,
                    )
                nc.scalar.copy(vn[bi][:, g4 * 4 : g4 * 4 + 4, :], pv)

            def emit_rot_vt(bi, pr_pool, tt_pool):
                """In-place RoPE + V transpose. k/q0 chunks interleaved
                (B consumes k and q0 first); q1 chunks alternate with vT
                groups as PE filler while the DVE chain catches up."""
                k_ap = kv_st[bi][:, 0, :]
                for ch in range(4):
                    emit_rot_chunk(bi, k_ap, 0, ch, pr_pool, tt_pool)
                    emit_rot_chunk(bi, q_st[bi][:, 0, :], 1, ch,
                                   pr_pool, tt_pool)
                for ch in range(4):
                    emit_rot_chunk(bi, q_st[bi][:, 1, :], 2, ch,
                                   pr_pool, tt_pool)
                    emit_vt_group(bi, ch, pr_pool)

            def emit_b_unit(bi, m, pools, fillers=None, micro=None,
                            micro_rate=2, sqg_hook=None):
                """One (batch, head) attention unit: 4 sqg of 8 sk-pairs.
                micro: queue of small PE tasks drained micro_rate per sk-pair
                (fine-grained interleave); fillers: one big task per sqg."""
                psc, pout, psum2, expool, recpool = pools
                for sqg in range(4):
                    qsl = slice(sqg * 512, (sqg + 1) * 512)
                    out_ps = pout.tile([P, 512], F32, tag="out",
                                       name=f"out{bi}{m}{sqg}")
                    sum_ps = psum2.tile([P, 512], F32, tag="sum",
                                        name=f"sum{bi}{m}{sqg}")
                    hsums = {}
                    for g in range(NSK // 2):
                        sc2 = psc.tile([P, 2, 512], F32, tag="sc",
                                       name=f"sc{bi}{m}{sqg}{g}")
                        for j in range(2):
                            sk = 2 * g + j
                            nc.tensor.matmul(
                                sc2[:, j, :],
                                kv_st[bi][:, 0, sk * P : (sk + 1) * P],
                                q_st[bi][:, m, qsl],
                                start=True, stop=True,
                            )
                        if add_mask:
                            for j in range(2):
                                nc.vector.scalar_tensor_tensor(
                                    sc2[:, j, :], sc2[:, j, :], SCALE,
                                    mask_sb[:, 2 * g + j, qsl],
                                    op0=mybir.AluOpType.mult,
                                    op1=mybir.AluOpType.add,
                                )
                        ex2 = expool.tile([P, 2, 512], BF16, tag="ex", bufs=3,
                                          name=f"ex{bi}{m}{sqg}{g}")
                        if add_mask:
                            nc.scalar.activation(
                                ex2, sc2, mybir.ActivationFunctionType.Exp,
                            )
                        else:
                            nc.scalar.activation(
                                ex2, sc2, mybir.ActivationFunctionType.Exp,
                                scale=SCALE,
                            )
                        for j in range(2):
                            sk = 2 * g + j
                            st_, sp_ = sk == 0, sk == NSK - 1
                            nc.tensor.matmul(
                                out_ps, vn[bi][:, sk, :], ex2[:, j, :],
                                start=st_, stop=sp_,
                            )
                        # softmax-sum 4:1 pre-reduction on DVE: the ones
                        # matmul count drops 16 -> 4 per sqg
                        h = expool.tile([P, 512], BF16, tag="hs", bufs=4,
                                        name=f"hs{bi}{m}{sqg}{g}")
                        nc.vector.tensor_add(h, ex2[:, 0, :], ex2[:, 1, :])
                        hsums[g] = h
                        if g % 2 == 1:
                            gg = g // 2
                            qs = expool.tile([P, 512], BF16, tag="qs", bufs=3,
                                             name=f"qs{bi}{m}{sqg}{gg}")
                            nc.vector.tensor_add(qs, hsums[g - 1], hsums[g])
                            hsums[g] = qs  # keep slot alive via dict
                            nc.tensor.matmul(
                                sum_ps, ones_sb, qs,
                                start=gg == 0, stop=gg == 3,
                            )
                        if micro:
                            for _ in range(micro_rate):
                                if micro:
                                    micro.pop(0)()
                    rec = recpool.tile([P, 512], F32, tag="rec", bufs=2,
                                       name=f"rec{bi}{m}{sqg}")
                    nc.vector.reciprocal_approx_fast(rec, sum_ps)
                    nc.vector.tensor_mul(outn[bi][:, m, qsl], out_ps, rec)
                    if fillers:
                        fillers.pop(0)()
                    if sqg_hook is not None:
                        sqg_hook(sqg)

            def emit_c_sqt(bi, sqt, po_pool, ob_pool):
                """o_proj for one 128-row seq block."""
                ob = ob_pool.tile([P, HIDDEN], BF16, tag="ob", bufs=3,
                                  name=f"ob{bi}{sqt}")
                ssl = slice(sqt * P, (sqt + 1) * P)
                for half in range(2):
                    po = po_pool.tile([P, 2, 512], F32, tag="po",
                                      name=f"po{bi}{sqt}{half}")
                    for hc in range(2):
                        col = (half * 2 + hc) * 512
                        for dc in range(NH_LOC):
                            nc.tensor.matmul(
                                po[:, hc, :],
                                outn[bi][:, dc, ssl],
                                wo_sb[:, dc, col : col + 512],
                                start=dc == 0, stop=dc == NH_LOC - 1,
                            )
                    osl = slice(half * 1024, (half + 1) * 1024)
                    if half == 0:
                        nc.scalar.copy(ob[:, osl], po)
                    else:
                        nc.vector.tensor_copy(ob[:, osl], po)
                nc.sync.dma_start(
                    out=out_d[bi * S + sqt * P : bi * S + (sqt + 1) * P, :],
                    in_=ob,
                )

            def emit_rot_vt_window(bi, w, pr_pool, tt_pool):
                """RoPE + vT for one 512-col window (window == rope chunk
                == vT group); emitted one window behind the projections so
                the PSUM drain is already done."""
                emit_rot_chunk(bi, kv_st[bi][:, 0, :], 0, w, pr_pool, tt_pool)
                emit_rot_chunk(bi, q_st[bi][:, 0, :], 1, w, pr_pool, tt_pool)
                emit_rot_chunk(bi, q_st[bi][:, 1, :], 2, w, pr_pool, tt_pool)
                emit_vt_group(bi, w, pr_pool)

            # ------------- the program -------------
            # A(b0): projection passes with window-local rope/vT trailing
            # one window behind (psA 4 banks + pr 2 + pv 2 = 8)
            psA_cm = tc.tile_pool(name="psA", bufs=2, space="PSUM")
            psA = psA_cm.__enter__()
            rv_cm = tc.tile_pool(name="rv0", bufs=2, space="PSUM")
            rv = rv_cm.__enter__()
            for w in range(4):
                emit_proj_pass(psA, 0, w, "kv")
                emit_proj_pass(psA, 0, w, "q")
                if w >= 1:
                    emit_rot_vt_window(0, w - 1, rv, stage)
            emit_rot_vt_window(0, 3, rv, stage)
            rv_cm.__exit__(None, None, None)
            psA_cm.__exit__(None, None, None)

            # B(b0,*) with proj(b1) passes as fillers
            fill_b1 = []
            psF_cm = tc.tile_pool(name="psF", bufs=1, space="PSUM")
            psF = psF_cm.__enter__()
            for w in range(4):
                for which in ("kv", "q"):
                    fill_b1.append(
                        lambda w=w, wh=which: emit_proj_pass(psF, 1, w, wh)
                    )

            for m in range(NH_LOC):
                pools_cm = [
                    tc.tile_pool(name=f"psc0{m}", bufs=2, space="PSUM"),
                    tc.tile_pool(name=f"pout0{m}", bufs=1, space="PSUM"),
                    tc.tile_pool(name=f"psum0{m}", bufs=1, space="PSUM"),
                    tc.tile_pool(name=f"ex0{m}", bufs=3),
                    tc.tile_pool(name=f"rec0{m}", bufs=2),
                ]
                pools = [p.__enter__() for p in pools_cm]
                emit_b_unit(0, m, pools, fillers=fill_b1)
                for p in reversed(pools_cm):
                    p.__exit__(None, None, None)
            psF_cm.__exit__(None, None, None)

            # rot + vT for b1
            rv1_cm = tc.tile_pool(name="rv1", bufs=2, space="PSUM")
            rv1 = rv1_cm.__enter__()
            emit_rot_vt(1, rv1, stage)
            rv1_cm.__exit__(None, None, None)

            # B(b1,*) with C(b0) as 2-sqt fillers; C(b1) dense tail
            poF_cm = tc.tile_pool(name="poF", bufs=1, space="PSUM")
            poF = poF_cm.__enter__()
            obF_cm = tc.tile_pool(name="obF", bufs=3)
            obF = obF_cm.__enter__()
            fill_c0 = []
            for pair in range(8):
                def filler(pair=pair):
                    emit_c_sqt(0, 2 * pair, poF, obF)
                    emit_c_sqt(0, 2 * pair + 1, poF, obF)
                fill_c0.append(filler)

            for m in range(NH_LOC):
                pools_cm = [
                    tc.tile_pool(name=f"psc1{m}", bufs=2, space="PSUM"),
                    tc.tile_pool(name=f"pout1{m}", bufs=1, space="PSUM"),
                    tc.tile_pool(name=f"psum1{m}", bufs=1, space="PSUM"),
                    tc.tile_pool(name=f"ex1{m}", bufs=3),
                    tc.tile_pool(name=f"rec1{m}", bufs=2),
                ]
                pools = [p.__enter__() for p in pools_cm]
                emit_b_unit(1, m, pools, fillers=fill_c0)
                for p in reversed(pools_cm):
                    p.__exit__(None, None, None)
            obF_cm.__exit__(None, None, None)
            poF_cm.__exit__(None, None, None)

            # C(b1) tail
            poT_cm = tc.tile_pool(name="poT", bufs=3, space="PSUM")
            poT = poT_cm.__enter__()
            obT_cm = tc.tile_pool(name="obT", bufs=3)
            obT = obT_cm.__enter__()
            for sqt in range(S // P):
                emit_c_sqt(1, sqt, poT, obT)
            obT_cm.__exit__(None, None, None)
            poT_cm.__exit__(None, None, None)
    nc.compile()
    return nc


_BUILD_CACHE = {}
LAST_RESULT = None


def _get_nc(add_mask):
    if add_mask not in _BUILD_CACHE:
        _BUILD_CACHE[add_mask] = _build(add_mask)
    return _BUILD_CACHE[add_mask]


def kernel(hidden_states, attention_mask, Wq, Wk, Wv, Wo):
    hidden_states = np.asarray(hidden_states, dtype=np.float32)
    attention_mask = np.asarray(attention_mask, dtype=np.float32)
    Wq = np.asarray(Wq, dtype=np.float32)
    Wk = np.asarray(Wk, dtype=np.float32)
    Wv = np.asarray(Wv, dtype=np.float32)
    Wo = np.asarray(Wo, dtype=np.float32)

    b, s, hidden = hidden_states.shape
    assert (b, s, hidden) == (B, S, HIDDEN)

    add_mask = bool(np.any(attention_mask))
    nc = _get_nc(add_mask)

    bf16 = ml_dtypes.bfloat16

    # X^T packed [p, w, c, s512]: hidden = c*128+p, seq-global = w*512+s
    xt = hidden_states.reshape(b * s, hidden).T  # [2048, 4096]
    xtb = np.ascontiguousarray(
        xt.reshape(KH, P, NW, 512).transpose(1, 2, 0, 3)
    ).astype(bf16)

    cos_t, sin_t = _rope_tables(s, HEAD_DIM, ROPE_THETA)
    cosb = cos_t.astype(bf16)
    rt = _rot_matrix_t(P).astype(bf16)
    ident = np.eye(P, dtype=np.float32).astype(bf16)
    onesb = np.ones((P, P), dtype=np.float32).astype(bf16)

    in_maps = []
    for c in range(N_CORES):
        kv = c // 2
        wq_c = Wq[:, c * NH_LOC * HEAD_DIM : (c + 1) * NH_LOC * HEAD_DIM]
        wk_c = Wk[:, kv * HEAD_DIM : (kv + 1) * HEAD_DIM]
        wv_c = Wv[:, kv * HEAD_DIM : (kv + 1) * HEAD_DIM]
        wo_c = Wo[c * NH_LOC * HEAD_DIM : (c + 1) * NH_LOC * HEAD_DIM, :]
        im = {
            "xtb": xtb,
            "wqb": np.ascontiguousarray(
                wq_c.reshape(KH, P, NH_LOC * P).transpose(1, 0, 2)
            ).astype(bf16),
            "wkb": np.ascontiguousarray(
                wk_c.reshape(KH, P, P).transpose(1, 0, 2)
            ).astype(bf16),
            "wvb": np.ascontiguousarray(
                wv_c.reshape(KH, P, P).transpose(1, 0, 2)
            ).astype(bf16),
            "wob": np.ascontiguousarray(
                wo_c.reshape(NH_LOC, P, HIDDEN).transpose(1, 0, 2)
            ).astype(bf16),
            "onesb": onesb,
            "cosb": cosb,
            "sinf": sin_t,
            "rt": rt,
            "ident": ident,
        }
        if add_mask:
            im["mask_t"] = np.ascontiguousarray(attention_mask[0, 0].T)
        in_maps.append(im)

    res = run_bass_kernel_spmd(nc, in_maps, core_ids=list(range(N_CORES)))
    global LAST_RESULT
    LAST_RESULT = res
    out = np.zeros((b * s, hidden), dtype=np.float32)
    for r in res.results:
        out += np.asarray(r["out"], dtype=np.float32)
    return out.reshape(b, s, hidden)


# revision 16
# speedup vs baseline: 1.3874x; 1.0335x over previous
"""GQA attention (dense_transformer) on 8 TRN2 NeuronCores.

Sharding: tensor-parallel over heads. Core c computes q-heads {2c, 2c+1}
(their shared kv head is c//2): column-parallel Wq/Wk/Wv, row-parallel Wo;
the 8 partial o_proj outputs are summed on the host.

v3 design (vs the f32r baseline):
  - all matmul operands bf16 (fp8 propagates ~3% element error straight
    to the output through the random-sign dot products here; bf16 keeps
    the stack at ~0.5%). PSUM accumulation stays fp32.
  - exp emitted 1024-wide ([sk-pair, sq] PSUM groups) straight to bf16.
  - RoPE applied in place (q_st/kv_st double as the roped tensors).
  - X^T streamed per 512-seq window (triple buffered), weights resident.
  - phase interleave: proj(b1) passes fill PE slack inside B(b0,*)'s
    sqg loop; C(b0) fills B(b1,*); only C(b1) trails.
  - PSUM budgeted <=8 banks in every region (2-bank proj passes).
"""

import math

import ml_dtypes
import numpy as np

import concourse.bacc as bacc_mod
import concourse.mybir as mybir
import concourse.tile as tile
from concourse.bass_utils import run_bass_kernel_spmd

HIDDEN = 2048
N_HEADS = 16
N_KV_HEADS = 4
HEAD_DIM = 128
ROPE_THETA = 10000.0
B = 2
S = 2048
N_CORES = 8
NH_LOC = N_HEADS // N_CORES  # 2 q heads per core
P = 128
F32 = mybir.dt.float32
BF16 = mybir.dt.bfloat16
SCALE = 1.0 / math.sqrt(HEAD_DIM)

KH = HIDDEN // P  # 16 contraction chunks
NW = B * 4  # 8 seq windows of 512
NSK = S // P  # 16 sk chunks


def _rope_tables(s, d, theta):
    inv_freq = 1.0 / (theta ** (np.arange(0, d, 2, dtype=np.float32) / d))
    t = np.arange(s, dtype=np.float32)
    freqs = np.outer(t, inv_freq).astype(np.float32)  # [S, d/2]
    emb = np.concatenate([freqs, freqs], axis=-1)  # [S, d]
    cos_t = np.ascontiguousarray(np.cos(emb).astype(np.float32).T)  # [d, S]
    sin_t = np.ascontiguousarray(np.sin(emb).astype(np.float32).T)
    return cos_t, sin_t


def _rot_matrix_t(d):
    # R @ q == rotate_half(q); stationary operand is R^T (matmul computes
    # lhsT.T @ rhs).
    r = np.zeros((d, d), dtype=np.float32)
    h = d // 2
    for i in range(h):
        r[i, i + h] = -1.0
        r[i + h, i] = 1.0
    return np.ascontiguousarray(r.T)


def _build(add_mask):
    nc = bacc_mod.Bacc()
    xt_d = nc.dram_tensor("xtb", [P, NW, KH, 512], BF16, kind="ExternalInput")
    wq_d = nc.dram_tensor("wqb", [P, KH, NH_LOC * P], BF16, kind="ExternalInput")
    wk_d = nc.dram_tensor("wkb", [P, KH, P], BF16, kind="ExternalInput")
    wv_d = nc.dram_tensor("wvb", [P, KH, P], BF16, kind="ExternalInput")
    wo_d = nc.dram_tensor("wob", [P, NH_LOC, HIDDEN], BF16, kind="ExternalInput")
    ones_d = nc.dram_tensor("onesb", [P, P], BF16, kind="ExternalInput")
    cosb_d = nc.dram_tensor("cosb", [P, S], BF16, kind="ExternalInput")
    sinf_d = nc.dram_tensor("sinf", [P, S], F32, kind="ExternalInput")
    rt_d = nc.dram_tensor("rt", [P, P], BF16, kind="ExternalInput")
    id_d = nc.dram_tensor("ident", [P, P], BF16, kind="ExternalInput")
    if add_mask:
        mt_d = nc.dram_tensor("mask_t", [S, S], F32, kind="ExternalInput")
    out_d = nc.dram_tensor("out", [B * S, HIDDEN], BF16, kind="ExternalOutput")

    with tile.TileContext(nc) as tc:
        with (
            tc.tile_pool(name="consts", bufs=1) as consts,
            tc.tile_pool(name="persist", bufs=1) as persist,
            tc.tile_pool(name="stage", bufs=1) as stage,
            tc.tile_pool(name="xstage", bufs=3) as xstage,
        ):
            # ---- persistent SBUF ----
            wq_sb = persist.tile([P, KH, NH_LOC * P], BF16, tag="wq")
            wk_sb = persist.tile([P, KH, P], BF16, tag="wk")
            wv_sb = persist.tile([P, KH, P], BF16, tag="wv")
            wo_sb = persist.tile([P, NH_LOC, HIDDEN], BF16, tag="wo")
            ones_sb = consts.tile([P, P], BF16, tag="ones")
            cos_sb = consts.tile([P, S], BF16, tag="cos")
            sin_sb = consts.tile([P, S], F32, tag="sin")
            rt_sb = consts.tile([P, P], BF16, tag="rt")
            id_sb = consts.tile([P, P], BF16, tag="id")

            # q_st/kv_st are roped in place; [:,0,:] of kv_st is k, [:,1,:] v
            q_st = [persist.tile([P, NH_LOC, S], BF16, tag=f"qst{bi}",
                                 name=f"qst{bi}") for bi in range(B)]
            kv_st = [persist.tile([P, 2, S], BF16, tag=f"kvst{bi}",
                                  name=f"kvst{bi}") for bi in range(B)]
            vn = [persist.tile([P, NSK, P], BF16, tag=f"vn{bi}",
                               name=f"vn{bi}") for bi in range(B)]
            outn = [persist.tile([P, NH_LOC, S], BF16, tag=f"on{bi}",
                                 name=f"on{bi}") for bi in range(B)]

            # ---- input DMAs: kv weights + first xt window lead so the
            # first (kv) projection pass starts as early as possible ----
            xw0 = xstage.tile([P, KH, 512], BF16, tag="xw", bufs=3,
                              name="xw0")
            nc.sync.dma_start(out=xw0[:, 0:4, :], in_=xt_d[:, 0, 0:4, :])
            nc.sync.dma_start(out=wk_sb, in_=wk_d[:, :, :])
            nc.sync.dma_start(out=wv_sb, in_=wv_d[:, :, :])
            # consts on the scalar (ACT) HWDGE queue — off the critical path
            nc.scalar.dma_start(out=cos_sb, in_=cosb_d[:, :])
            nc.scalar.dma_start(out=sin_sb, in_=sinf_d[:, :])
            nc.scalar.dma_start(out=rt_sb, in_=rt_d[:, :])
            nc.scalar.dma_start(out=id_sb, in_=id_d[:, :])
            nc.scalar.dma_start(out=ones_sb, in_=ones_d[:, :])
            nc.scalar.dma_start(out=wo_sb, in_=wo_d[:, :, :])
            if add_mask:
                mask_sb = persist.tile([P, NSK, S], F32, tag="mask")
                nc.scalar.dma_start(
                    out=mask_sb, in_=mt_d.rearrange("(c p) m -> p c m", p=P)
                )
            # prewarm the exp table set during phase A
            warm = stage.tile([P, 8], BF16, tag="warm")
            nc.scalar.activation(
                warm, cos_sb[:, :8], mybir.ActivationFunctionType.Exp
            )

            # xt windows, streamed + triple buffered
            xw_tiles = {}

            def get_xw(w):
                if w not in xw_tiles:
                    t = xstage.tile([P, KH, 512], BF16, tag="xw", bufs=3,
                                    name=f"xw{w}")
                    nc.sync.dma_start(out=t, in_=xt_d[:, w])
                    xw_tiles[w] = t
                return xw_tiles[w]

            # rest of the first window + q weights
            for cq in range(1, 4):
                nc.sync.dma_start(
                    out=xw0[:, 4 * cq : 4 * cq + 4, :],
                    in_=xt_d[:, 0, 4 * cq : 4 * cq + 4, :],
                )
            xw_tiles[0] = xw0
            nc.sync.dma_start(out=wq_sb, in_=wq_d[:, :, :])

            # ------------- emission helpers -------------
            def emit_proj_pass(pool, bi, w, which):
                """One 2-bank projection pass: 32 matmuls + 1 drain."""
                pp = pool.tile([P, 2, 512], F32, tag="pp",
                               name=f"pp{bi}{w}{which}")
                xw = get_xw(bi * 4 + w)
                for c in range(KH):
                    st_, sp_ = c == 0, c == KH - 1
                    if which == "q":
                        nc.tensor.matmul(
                            pp[:, 0, :], wq_sb[:, c, 0:P], xw[:, c, :],
                            start=st_, stop=sp_,
                        )
                        nc.tensor.matmul(
                            pp[:, 1, :], wq_sb[:, c, P : 2 * P], xw[:, c, :],
                            start=st_, stop=sp_,
                        )
                    else:
                        nc.tensor.matmul(
                            pp[:, 0, :], wk_sb[:, c, :], xw[:, c, :],
                            start=st_, stop=sp_,
                        )
                        nc.tensor.matmul(
                            pp[:, 1, :], wv_sb[:, c, :], xw[:, c, :],
                            start=st_, stop=sp_,
                        )
                dst = q_st[bi] if which == "q" else kv_st[bi]
                sl = slice(w * 512, (w + 1) * 512)
                nc.scalar.copy(dst[:, :, sl], pp)

            def emit_rot_chunk(bi, src_ap, ji, ch, pr_pool, tt_pool):
                sl = slice(ch * 512, (ch + 1) * 512)
                pr = pr_pool.tile([P, 512], F32, tag="pr", bufs=2,
                                  name=f"pr{bi}{ji}{ch}")
                nc.tensor.matmul(pr, rt_sb, src_ap[:, sl],
                                 start=True, stop=True)
                t_t = tt_pool.tile([P, 512], BF16, tag="tt", bufs=4,
                                   name=f"tt{bi}{ji}{ch}")
                nc.vector.tensor_mul(t_t, pr, sin_sb[:, sl])
                x_t = tt_pool.tile([P, 512], BF16, tag="xt2", bufs=4,
                                   name=f"xt2{bi}{ji}{ch}")
                nc.vector.tensor_mul(x_t, src_ap[:, sl], cos_sb[:, sl])
                nc.vector.tensor_add(src_ap[:, sl], x_t, t_t)

            def emit_vt_group(bi, g4, pv_pool):
                pv = pv_pool.tile([P, 512], BF16, tag="pv", bufs=2,
                                  name=f"pv{bi}{g4}")
                for j in range(4):
                    blk = g4 * 4 + j
                    nc.tensor.matmul(
                        pv[:, j * P : (j + 1) * P],
                        kv_st[bi][:, 1, blk * P : (blk + 1) * P],
                        id_sb, is_transpose=True, start=True, stop=True,
                    )
                nc.scalar.copy(vn[bi][:, g4 * 4 : g4 * 4 + 4, :], pv)

            def emit_rot_vt(bi, pr_pool, tt_pool):
                """In-place RoPE + V transpose. k/q0 chunks interleaved
                (B consumes k and q0 first); q1 chunks alternate with vT
                groups as PE filler while the DVE chain catches up."""
                k_ap = kv_st[bi][:, 0, :]
                for ch in range(4):
                    emit_rot_chunk(bi, k_ap, 0, ch, pr_pool, tt_pool)
                    emit_rot_chunk(bi, q_st[bi][:, 0, :], 1, ch,
                                   pr_pool, tt_pool)
                for ch in range(4):
                    emit_rot_chunk(bi, q_st[bi][:, 1, :], 2, ch,
                                   pr_pool, tt_pool)
                    emit_vt_group(bi, ch, pr_pool)

            def emit_b_unit(bi, m, pools, fillers=None, micro=None,
                            micro_rate=2, sqg_hook=None):
                """One (batch, head) attention unit: 4 sqg of 8 sk-pairs.
                micro: queue of small PE tasks drained micro_rate per sk-pair
                (fine-grained interleave); fillers: one big task per sqg."""
                psc, pout, psum2, expool, recpool = pools
                for sqg in range(4):
                    qsl = slice(sqg * 512, (sqg + 1) * 512)
                    out_ps = pout.tile([P, 512], F32, tag="out",
                                       name=f"out{bi}{m}{sqg}")
                    sum_ps = psum2.tile([P, 512], F32, tag="sum",
                                        name=f"sum{bi}{m}{sqg}")
                    hsums = {}
                    for g in range(NSK // 2):
                        sc2 = psc.tile([P, 1024], F32, tag="sc",
                                       name=f"sc{bi}{m}{sqg}{g}")
                        for j in range(2):
                            sk = 2 * g + j
                            nc.tensor.matmul(
                                sc2[:, j * 512 : (j + 1) * 512],
                                kv_st[bi][:, 0, sk * P : (sk + 1) * P],
                                q_st[bi][:, m, qsl],
                                start=True, stop=True,
                            )
                        if add_mask:
                            for j in range(2):
                                nc.vector.scalar_tensor_tensor(
                                    sc2[:, j * 512 : (j + 1) * 512],
                                    sc2[:, j * 512 : (j + 1) * 512], SCALE,
                                    mask_sb[:, 2 * g + j, qsl],
                                    op0=mybir.AluOpType.mult,
                                    op1=mybir.AluOpType.add,
                                )
                        ex2 = expool.tile([P, 1024], BF16, tag="ex", bufs=3,
                                          name=f"ex{bi}{m}{sqg}{g}")
                        if add_mask:
                            nc.scalar.activation(
                                ex2, sc2, mybir.ActivationFunctionType.Exp,
                            )
                        else:
                            nc.scalar.activation(
                                ex2, sc2, mybir.ActivationFunctionType.Exp,
                                scale=SCALE,
                            )
                        for j in range(2):
                            sk = 2 * g + j
                            st_, sp_ = sk == 0, sk == NSK - 1
                            nc.tensor.matmul(
                                out_ps, vn[bi][:, sk, :],
                                ex2[:, j * 512 : (j + 1) * 512],
                                start=st_, stop=sp_,
                            )
                        # softmax-sum 4:1 pre-reduction on DVE: the ones
                        # matmul count drops 16 -> 4 per sqg
                        h = expool.tile([P, 512], BF16, tag="hs", bufs=4,
                                        name=f"hs{bi}{m}{sqg}{g}")
                        nc.vector.tensor_add(h, ex2[:, 0:512],
                                             ex2[:, 512:1024])
                        hsums[g] = h
                        if g % 2 == 1:
                            gg = g // 2
                            qs = expool.tile([P, 512], BF16, tag="qs", bufs=3,
                                             name=f"qs{bi}{m}{sqg}{gg}")
                            nc.vector.tensor_add(qs, hsums[g - 1], hsums[g])
                            hsums[g] = qs  # keep slot alive via dict
                            nc.tensor.matmul(
                                sum_ps, ones_sb, qs,
                                start=gg == 0, stop=gg == 3,
                            )
                        if micro:
                            for _ in range(micro_rate):
                                if micro:
                                    micro.pop(0)()
                    rec = recpool.tile([P, 512], F32, tag="rec", bufs=2,
                                       name=f"rec{bi}{m}{sqg}")
                    nc.vector.reciprocal_approx_fast(rec, sum_ps)
                    nc.vector.tensor_mul(outn[bi][:, m, qsl], out_ps, rec)
                    if fillers:
                        fillers.pop(0)()
                    if sqg_hook is not None:
                        sqg_hook(sqg)

            def emit_c_sqt(bi, sqt, po_pool, ob_pool):
                """o_proj for one 128-row seq block: 4 single-bank psum
                steps, drains alternating between ACT and DVE."""
                ob = ob_pool.tile([P, HIDDEN], BF16, tag="ob", bufs=3,
                                  name=f"ob{bi}{sqt}")
                ssl = slice(sqt * P, (sqt + 1) * P)
                for hc in range(4):
                    po = po_pool.tile([P, 512], F32, tag="po", bufs=2,
                                      name=f"po{bi}{sqt}{hc}")
                    col = hc * 512
                    for dc in range(NH_LOC):
                        nc.tensor.matmul(
                            po,
                            outn[bi][:, dc, ssl],
                            wo_sb[:, dc, col : col + 512],
                            start=dc == 0, stop=dc == NH_LOC - 1,
                        )
                    osl = slice(col, col + 512)
                    if hc % 2 == 0:
                        nc.scalar.copy(ob[:, osl], po)
                    else:
                        nc.vector.tensor_copy(ob[:, osl], po)
                nc.sync.dma_start(
                    out=out_d[bi * S + sqt * P : bi * S + (sqt + 1) * P, :],
                    in_=ob,
                )

            def emit_rot_vt_window(bi, w, pr_pool, tt_pool):
                """RoPE + vT for one 512-col window (window == rope chunk
                == vT group); emitted one window behind the projections so
                the PSUM drain is already done."""
                emit_rot_chunk(bi, kv_st[bi][:, 0, :], 0, w, pr_pool, tt_pool)
                emit_rot_chunk(bi, q_st[bi][:, 0, :], 1, w, pr_pool, tt_pool)
                emit_rot_chunk(bi, q_st[bi][:, 1, :], 2, w, pr_pool, tt_pool)
                emit_vt_group(bi, w, pr_pool)

            # ------------- the program -------------
            # A(b0): projection passes with window-local rope/vT trailing
            # one window behind (psA 4 banks + pr 2 + pv 2 = 8)
            psA_cm = tc.tile_pool(name="psA", bufs=2, space="PSUM")
            psA = psA_cm.__enter__()
            rv_cm = tc.tile_pool(name="rv0", bufs=2, space="PSUM")
            rv = rv_cm.__enter__()
            for w in range(4):
                emit_proj_pass(psA, 0, w, "kv")
                emit_proj_pass(psA, 0, w, "q")
                if w >= 1:
                    emit_rot_vt_window(0, w - 1, rv, stage)
            emit_rot_vt_window(0, 3, rv, stage)
            rv_cm.__exit__(None, None, None)
            psA_cm.__exit__(None, None, None)

            # B(b0,*): proj(b1) passes as fillers; h1's boundaries also
            # carry the k/q0 rope of b1 through spare psc slots
            fill_b1 = []
            psF_cm = tc.tile_pool(name="psF", bufs=1, space="PSUM")
            psF = psF_cm.__enter__()
            for w in range(4):
                for which in ("kv", "q"):
                    fill_b1.append(
                        lambda w=w, wh=which: emit_proj_pass(psF, 1, w, wh)
                    )

            def rope_pair_hook(psc_pool, jobs):
                """Two in-place rope chunks through one spare sc slot."""
                def hook(sqg, jobs=jobs):
                    pair = jobs.pop(0) if jobs else None
                    if pair is None:
                        return
                    sct = psc_pool.tile([P, 1024], F32, tag="sc",
                                        name=f"rp{pair[0][1]}{pair[0][2]}")
                    for half, (src_ap, ji, ch) in enumerate(pair):
                        sl = slice(ch * 512, (ch + 1) * 512)
                        pr = sct[:, half * 512 : (half + 1) * 512]
                        nc.tensor.matmul(pr, rt_sb, src_ap[:, sl],
                                         start=True, stop=True)
                        t_t = stage.tile([P, 512], BF16, tag="tt", bufs=4,
                                         name=f"tt1{ji}{ch}")
                        nc.vector.tensor_mul(t_t, pr, sin_sb[:, sl])
                        x_t = stage.tile([P, 512], BF16, tag="xt2", bufs=4,
                                         name=f"xt21{ji}{ch}")
                        nc.vector.tensor_mul(x_t, src_ap[:, sl],
                                             cos_sb[:, sl])
                        nc.vector.tensor_add(src_ap[:, sl], x_t, t_t)
                return hook

            k1_ap = kv_st[1][:, 0, :]
            q10_ap = q_st[1][:, 0, :]
            # boundary i's chunks only touch windows whose projection
            # filler has already been emitted (h0: w0/w1, h1 bN: see map)
            rope_b1_pairs = [
                [(k1_ap, 0, 0), (k1_ap, 0, 1)],
                [(k1_ap, 0, 2), (q10_ap, 1, 0)],
                [(k1_ap, 0, 3), (q10_ap, 1, 1)],
                [(q10_ap, 1, 2), (q10_ap, 1, 3)],
            ]

            for m in range(NH_LOC):
                pools_cm = [
                    tc.tile_pool(name=f"psc0{m}", bufs=2, space="PSUM"),
                    tc.tile_pool(name=f"pout0{m}", bufs=1, space="PSUM"),
                    tc.tile_pool(name=f"psum0{m}", bufs=1, space="PSUM"),
                    tc.tile_pool(name=f"ex0{m}", bufs=3),
                    tc.tile_pool(name=f"rec0{m}", bufs=2),
                ]
                pools = [p.__enter__() for p in pools_cm]
                hook = (rope_pair_hook(pools[0], rope_b1_pairs)
                        if m == 1 else None)
                emit_b_unit(0, m, pools, fillers=fill_b1, sqg_hook=hook)
                for p in reversed(pools_cm):
                    p.__exit__(None, None, None)
            psF_cm.__exit__(None, None, None)

            # vT(b1) + q1 rope of b1 (k/q0 were roped inside B(b0,h1))
            rv1_cm = tc.tile_pool(name="rv1", bufs=2, space="PSUM")
            rv1 = rv1_cm.__enter__()
            for g4 in range(4):
                emit_vt_group(1, g4, rv1)
            for ch in range(4):
                emit_rot_chunk(1, q_st[1][:, 1, :], 2, ch, rv1, stage)
            rv1_cm.__exit__(None, None, None)

            # B(b1,*): h0 carries C(b0) 2-sqt fillers (2 per boundary);
            # h1 carries window-gated C(b1) fillers; short dense tail
            poF_cm = tc.tile_pool(name="poF", bufs=2, space="PSUM")
            poF = poF_cm.__enter__()
            obF_cm = tc.tile_pool(name="obF", bufs=3)
            obF = obF_cm.__enter__()

            fill_c0 = []
            for quad in range(4):
                def filler(quad=quad):
                    for k in range(4):
                        emit_c_sqt(0, 4 * quad + k, poF, obF)
                fill_c0.append(filler)

            c1_next = [0]

            def c1_hook(sqg):
                # outn[1] window sqg complete -> rows 4*sqg..4*sqg+3 legal
                hi = min(4 * (sqg + 1), S // P)
                n = min(4, hi - c1_next[0])
                for _ in range(n):
                    emit_c_sqt(1, c1_next[0], poF, obF)
                    c1_next[0] += 1

            for m in range(NH_LOC):
                pools_cm = [
                    tc.tile_pool(name=f"psc1{m}", bufs=2, space="PSUM"),
                    tc.tile_pool(name=f"pout1{m}", bufs=1, space="PSUM"),
                    tc.tile_pool(name=f"psum1{m}", bufs=1, space="PSUM"),
                    tc.tile_pool(name=f"ex1{m}", bufs=3),
                    tc.tile_pool(name=f"rec1{m}", bufs=2),
                ]
                pools = [p.__enter__() for p in pools_cm]
                if m == 0:
                    emit_b_unit(1, m, pools, fillers=fill_c0)
                else:
                    emit_b_unit(1, m, pools, sqg_hook=c1_hook)
                for p in reversed(pools_cm):
                    p.__exit__(None, None, None)

            # dense tail: remaining C(b1) rows
            while c1_next[0] < S // P:
                emit_c_sqt(1, c1_next[0], poF, obF)
                c1_next[0] += 1
            obF_cm.__exit__(None, None, None)
            poF_cm.__exit__(None, None, None)
    nc.compile()
    return nc


_BUILD_CACHE = {}
LAST_RESULT = None


def _get_nc(add_mask):
    if add_mask not in _BUILD_CACHE:
        _BUILD_CACHE[add_mask] = _build(add_mask)
    return _BUILD_CACHE[add_mask]


def kernel(hidden_states, attention_mask, Wq, Wk, Wv, Wo):
    hidden_states = np.asarray(hidden_states, dtype=np.float32)
    attention_mask = np.asarray(attention_mask, dtype=np.float32)
    Wq = np.asarray(Wq, dtype=np.float32)
    Wk = np.asarray(Wk, dtype=np.float32)
    Wv = np.asarray(Wv, dtype=np.float32)
    Wo = np.asarray(Wo, dtype=np.float32)

    b, s, hidden = hidden_states.shape
    assert (b, s, hidden) == (B, S, HIDDEN)

    add_mask = bool(np.any(attention_mask))
    nc = _get_nc(add_mask)

    bf16 = ml_dtypes.bfloat16

    # X^T packed [p, w, c, s512]: hidden = c*128+p, seq-global = w*512+s
    xt = hidden_states.reshape(b * s, hidden).T  # [2048, 4096]
    xtb = np.ascontiguousarray(
        xt.reshape(KH, P, NW, 512).transpose(1, 2, 0, 3)
    ).astype(bf16)

    cos_t, sin_t = _rope_tables(s, HEAD_DIM, ROPE_THETA)
    cosb = cos_t.astype(bf16)
    rt = _rot_matrix_t(P).astype(bf16)
    ident = np.eye(P, dtype=np.float32).astype(bf16)
    onesb = np.ones((P, P), dtype=np.float32).astype(bf16)

    in_maps = []
    for c in range(N_CORES):
        kv = c // 2
        wq_c = Wq[:, c * NH_LOC * HEAD_DIM : (c + 1) * NH_LOC * HEAD_DIM]
        wk_c = Wk[:, kv * HEAD_DIM : (kv + 1) * HEAD_DIM]
        wv_c = Wv[:, kv * HEAD_DIM : (kv + 1) * HEAD_DIM]
        wo_c = Wo[c * NH_LOC * HEAD_DIM : (c + 1) * NH_LOC * HEAD_DIM, :]
        im = {
            "xtb": xtb,
            "wqb": np.ascontiguousarray(
                wq_c.reshape(KH, P, NH_LOC * P).transpose(1, 0, 2)
            ).astype(bf16),
            "wkb": np.ascontiguousarray(
                wk_c.reshape(KH, P, P).transpose(1, 0, 2)
            ).astype(bf16),
            "wvb": np.ascontiguousarray(
                wv_c.reshape(KH, P, P).transpose(1, 0, 2)
            ).astype(bf16),
            "wob": np.ascontiguousarray(
                wo_c.reshape(NH_LOC, P, HIDDEN).transpose(1, 0, 2)
            ).astype(bf16),
            "onesb": onesb,
            "cosb": cosb,
            "sinf": sin_t,
            "rt": rt,
            "ident": ident,
        }
        if add_mask:
            im["mask_t"] = np.ascontiguousarray(attention_mask[0, 0].T)
        in_maps.append(im)

    res = run_bass_kernel_spmd(nc, in_maps, core_ids=list(range(N_CORES)))
    global LAST_RESULT
    LAST_RESULT = res
    out = np.zeros((b * s, hidden), dtype=np.float32)
    for r in res.results:
        out += np.asarray(r["out"], dtype=np.float32)
    return out.reshape(b, s, hidden)


# revision 17
# speedup vs baseline: 1.4186x; 1.0225x over previous
"""GQA attention (dense_transformer) on 8 TRN2 NeuronCores.

Sharding: tensor-parallel over heads. Core c computes q-heads {2c, 2c+1}
(their shared kv head is c//2): column-parallel Wq/Wk/Wv, row-parallel Wo;
the 8 partial o_proj outputs are summed on the host.

v3 design (vs the f32r baseline):
  - all matmul operands bf16 (fp8 propagates ~3% element error straight
    to the output through the random-sign dot products here; bf16 keeps
    the stack at ~0.5%). PSUM accumulation stays fp32.
  - exp emitted 1024-wide ([sk-pair, sq] PSUM groups) straight to bf16.
  - RoPE applied in place (q_st/kv_st double as the roped tensors).
  - X^T streamed per 512-seq window (triple buffered), weights resident.
  - phase interleave: proj(b1) passes fill PE slack inside B(b0,*)'s
    sqg loop; C(b0) fills B(b1,*); only C(b1) trails.
  - PSUM budgeted <=8 banks in every region (2-bank proj passes).
"""

import math

import ml_dtypes
import numpy as np

import concourse.bacc as bacc_mod
import concourse.mybir as mybir
import concourse.tile as tile
from concourse.bass_utils import run_bass_kernel_spmd

HIDDEN = 2048
N_HEADS = 16
N_KV_HEADS = 4
HEAD_DIM = 128
ROPE_THETA = 10000.0
B = 2
S = 2048
N_CORES = 8
NH_LOC = N_HEADS // N_CORES  # 2 q heads per core
P = 128
F32 = mybir.dt.float32
BF16 = mybir.dt.bfloat16
SCALE = 1.0 / math.sqrt(HEAD_DIM)

KH = HIDDEN // P  # 16 contraction chunks
NW = B * 4  # 8 seq windows of 512
NSK = S // P  # 16 sk chunks


def _rope_tables(s, d, theta):
    inv_freq = 1.0 / (theta ** (np.arange(0, d, 2, dtype=np.float32) / d))
    t = np.arange(s, dtype=np.float32)
    freqs = np.outer(t, inv_freq).astype(np.float32)  # [S, d/2]
    emb = np.concatenate([freqs, freqs], axis=-1)  # [S, d]
    cos_t = np.ascontiguousarray(np.cos(emb).astype(np.float32).T)  # [d, S]
    sin_t = np.ascontiguousarray(np.sin(emb).astype(np.float32).T)
    return cos_t, sin_t


def _rot_matrix_t(d):
    # R @ q == rotate_half(q); stationary operand is R^T (matmul computes
    # lhsT.T @ rhs).
    r = np.zeros((d, d), dtype=np.float32)
    h = d // 2
    for i in range(h):
        r[i, i + h] = -1.0
        r[i + h, i] = 1.0
    return np.ascontiguousarray(r.T)


def _build(add_mask):
    nc = bacc_mod.Bacc()
    xt_d = nc.dram_tensor("xtb", [P, NW, KH, 512], BF16, kind="ExternalInput")
    wq_d = nc.dram_tensor("wqb", [P, KH, NH_LOC * P], BF16, kind="ExternalInput")
    wk_d = nc.dram_tensor("wkb", [P, KH, P], BF16, kind="ExternalInput")
    wv_d = nc.dram_tensor("wvb", [P, KH, P], BF16, kind="ExternalInput")
    wo_d = nc.dram_tensor("wob", [P, NH_LOC, HIDDEN], BF16, kind="ExternalInput")
    ones_d = nc.dram_tensor("onesb", [P, P], BF16, kind="ExternalInput")
    cosb_d = nc.dram_tensor("cosb", [P, S], BF16, kind="ExternalInput")
    sinf_d = nc.dram_tensor("sinf", [P, S], F32, kind="ExternalInput")
    rt_d = nc.dram_tensor("rt", [P, P], BF16, kind="ExternalInput")
    id_d = nc.dram_tensor("ident", [P, P], BF16, kind="ExternalInput")
    if add_mask:
        mt_d = nc.dram_tensor("mask_t", [S, S], F32, kind="ExternalInput")
    out_d = nc.dram_tensor("out", [B * S, HIDDEN], BF16, kind="ExternalOutput")

    with tile.TileContext(nc) as tc:
        with (
            tc.tile_pool(name="consts", bufs=1) as consts,
            tc.tile_pool(name="persist", bufs=1) as persist,
            tc.tile_pool(name="stage", bufs=1) as stage,
            tc.tile_pool(name="xstage", bufs=3) as xstage,
        ):
            # ---- persistent SBUF ----
            wq_sb = persist.tile([P, KH, NH_LOC * P], BF16, tag="wq")
            wk_sb = persist.tile([P, KH, P], BF16, tag="wk")
            wv_sb = persist.tile([P, KH, P], BF16, tag="wv")
            wo_sb = persist.tile([P, NH_LOC, HIDDEN], BF16, tag="wo")
            ones_sb = consts.tile([P, P], BF16, tag="ones")
            cos_sb = consts.tile([P, S], BF16, tag="cos")
            sin_sb = consts.tile([P, S], F32, tag="sin")
            rt_sb = consts.tile([P, P], BF16, tag="rt")
            id_sb = consts.tile([P, P], BF16, tag="id")

            # q_st/kv_st are roped in place; [:,0,:] of kv_st is k, [:,1,:] v
            q_st = [persist.tile([P, NH_LOC, S], BF16, tag=f"qst{bi}",
                                 name=f"qst{bi}") for bi in range(B)]
            kv_st = [persist.tile([P, 2, S], BF16, tag=f"kvst{bi}",
                                  name=f"kvst{bi}") for bi in range(B)]
            vn = [persist.tile([P, NSK, P], BF16, tag=f"vn{bi}",
                               name=f"vn{bi}") for bi in range(B)]
            outn = [persist.tile([P, NH_LOC, S], BF16, tag=f"on{bi}",
                                 name=f"on{bi}") for bi in range(B)]

            # ---- input DMAs: kv weights + first xt window lead so the
            # first (kv) projection pass starts as early as possible ----
            xw0 = xstage.tile([P, KH, 512], BF16, tag="xw", bufs=3,
                              name="xw0")
            nc.sync.dma_start(out=xw0[:, 0:4, :], in_=xt_d[:, 0, 0:4, :])
            # weights + consts on the scalar (ACT) HWDGE queue, weights
            # first — the sync queue carries only the xt window stream
            nc.scalar.dma_start(out=wk_sb, in_=wk_d[:, :, :])
            nc.scalar.dma_start(out=wv_sb, in_=wv_d[:, :, :])
            nc.scalar.dma_start(out=wq_sb, in_=wq_d[:, :, :])
            nc.scalar.dma_start(out=cos_sb, in_=cosb_d[:, :])
            nc.scalar.dma_start(out=sin_sb, in_=sinf_d[:, :])
            nc.scalar.dma_start(out=rt_sb, in_=rt_d[:, :])
            nc.scalar.dma_start(out=id_sb, in_=id_d[:, :])
            nc.scalar.dma_start(out=ones_sb, in_=ones_d[:, :])
            nc.scalar.dma_start(out=wo_sb, in_=wo_d[:, :, :])
            if add_mask:
                mask_sb = persist.tile([P, NSK, S], F32, tag="mask")
                nc.scalar.dma_start(
                    out=mask_sb, in_=mt_d.rearrange("(c p) m -> p c m", p=P)
                )
            # prewarm the exp table set during phase A
            warm = stage.tile([P, 8], BF16, tag="warm")
            nc.scalar.activation(
                warm, cos_sb[:, :8], mybir.ActivationFunctionType.Exp
            )

            # xt windows, streamed + triple buffered
            xw_tiles = {}

            def get_xw(w):
                if w not in xw_tiles:
                    t = xstage.tile([P, KH, 512], BF16, tag="xw", bufs=3,
                                    name=f"xw{w}")
                    nc.sync.dma_start(out=t, in_=xt_d[:, w])
                    xw_tiles[w] = t
                return xw_tiles[w]

            # rest of the first window
            for cq in range(1, 4):
                nc.sync.dma_start(
                    out=xw0[:, 4 * cq : 4 * cq + 4, :],
                    in_=xt_d[:, 0, 4 * cq : 4 * cq + 4, :],
                )
            xw_tiles[0] = xw0

            # ------------- emission helpers -------------
            def emit_proj_pass(pool, bi, w, which):
                """One 2-bank projection pass: 32 matmuls + 1 drain."""
                pp = pool.tile([P, 2, 512], F32, tag="pp",
                               name=f"pp{bi}{w}{which}")
                xw = get_xw(bi * 4 + w)
                for c in range(KH):
                    st_, sp_ = c == 0, c == KH - 1
                    if which == "q":
                        nc.tensor.matmul(
                            pp[:, 0, :], wq_sb[:, c, 0:P], xw[:, c, :],
                            start=st_, stop=sp_,
                        )
                        nc.tensor.matmul(
                            pp[:, 1, :], wq_sb[:, c, P : 2 * P], xw[:, c, :],
                            start=st_, stop=sp_,
                        )
                    else:
                        nc.tensor.matmul(
                            pp[:, 0, :], wk_sb[:, c, :], xw[:, c, :],
                            start=st_, stop=sp_,
                        )
                        nc.tensor.matmul(
                            pp[:, 1, :], wv_sb[:, c, :], xw[:, c, :],
                            start=st_, stop=sp_,
                        )
                dst = q_st[bi] if which == "q" else kv_st[bi]
                sl = slice(w * 512, (w + 1) * 512)
                nc.scalar.copy(dst[:, :, sl], pp)

            def emit_rot_chunk(bi, src_ap, ji, ch, pr_pool, tt_pool):
                sl = slice(ch * 512, (ch + 1) * 512)
                pr = pr_pool.tile([P, 512], F32, tag="pr", bufs=2,
                                  name=f"pr{bi}{ji}{ch}")
                nc.tensor.matmul(pr, rt_sb, src_ap[:, sl],
                                 start=True, stop=True)
                t_t = tt_pool.tile([P, 512], BF16, tag="tt", bufs=4,
                                   name=f"tt{bi}{ji}{ch}")
                nc.vector.tensor_mul(t_t, pr, sin_sb[:, sl])
                x_t = tt_pool.tile([P, 512], BF16, tag="xt2", bufs=4,
                                   name=f"xt2{bi}{ji}{ch}")
                nc.vector.tensor_mul(x_t, src_ap[:, sl], cos_sb[:, sl])
                nc.vector.tensor_add(src_ap[:, sl], x_t, t_t)

            def emit_vt_group(bi, g4, pv_pool):
                pv = pv_pool.tile([P, 512], BF16, tag="pv", bufs=2,
                                  name=f"pv{bi}{g4}")
                for j in range(4):
                    blk = g4 * 4 + j
                    nc.tensor.matmul(
                        pv[:, j * P : (j + 1) * P],
                        kv_st[bi][:, 1, blk * P : (blk + 1) * P],
                        id_sb, is_transpose=True, start=True, stop=True,
                    )
                nc.scalar.copy(vn[bi][:, g4 * 4 : g4 * 4 + 4, :], pv)

            def emit_rot_vt(bi, pr_pool, tt_pool):
                """In-place RoPE + V transpose. k/q0 chunks interleaved
                (B consumes k and q0 first); q1 chunks alternate with vT
                groups as PE filler while the DVE chain catches up."""
                k_ap = kv_st[bi][:, 0, :]
                for ch in range(4):
                    emit_rot_chunk(bi, k_ap, 0, ch, pr_pool, tt_pool)
                    emit_rot_chunk(bi, q_st[bi][:, 0, :], 1, ch,
                                   pr_pool, tt_pool)
                for ch in range(4):
                    emit_rot_chunk(bi, q_st[bi][:, 1, :], 2, ch,
                                   pr_pool, tt_pool)
                    emit_vt_group(bi, ch, pr_pool)

            def emit_b_unit(bi, m, pools, fillers=None, micro=None,
                            micro_rate=2, sqg_hook=None):
                """One (batch, head) attention unit: 4 sqg of 8 sk-pairs.
                micro: queue of small PE tasks drained micro_rate per sk-pair
                (fine-grained interleave); fillers: one big task per sqg."""
                psc, pout, psum2, expool, recpool = pools
                for sqg in range(4):
                    qsl = slice(sqg * 512, (sqg + 1) * 512)
                    out_ps = pout.tile([P, 512], F32, tag="out",
                                       name=f"out{bi}{m}{sqg}")
                    sum_ps = psum2.tile([P, 512], F32, tag="sum",
                                        name=f"sum{bi}{m}{sqg}")
                    hsums = {}
                    for g in range(NSK // 2):
                        sc2 = psc.tile([P, 1024], F32, tag="sc",
                                       name=f"sc{bi}{m}{sqg}{g}")
                        for j in range(2):
                            sk = 2 * g + j
                            nc.tensor.matmul(
                                sc2[:, j * 512 : (j + 1) * 512],
                                kv_st[bi][:, 0, sk * P : (sk + 1) * P],
                                q_st[bi][:, m, qsl],
                                start=True, stop=True,
                            )
                        if add_mask:
                            for j in range(2):
                                nc.vector.scalar_tensor_tensor(
                                    sc2[:, j * 512 : (j + 1) * 512],
                                    sc2[:, j * 512 : (j + 1) * 512], SCALE,
                                    mask_sb[:, 2 * g + j, qsl],
                                    op0=mybir.AluOpType.mult,
                                    op1=mybir.AluOpType.add,
                                )
                        ex2 = expool.tile([P, 1024], BF16, tag="ex", bufs=3,
                                          name=f"ex{bi}{m}{sqg}{g}")
                        if add_mask:
                            nc.scalar.activation(
                                ex2, sc2, mybir.ActivationFunctionType.Exp,
                            )
                        else:
                            nc.scalar.activation(
                                ex2, sc2, mybir.ActivationFunctionType.Exp,
                                scale=SCALE,
                            )
                        for j in range(2):
                            sk = 2 * g + j
                            st_, sp_ = sk == 0, sk == NSK - 1
                            nc.tensor.matmul(
                                out_ps, vn[bi][:, sk, :],
                                ex2[:, j * 512 : (j + 1) * 512],
                                start=st_, stop=sp_,
                            )
                        # softmax-sum 4:1 pre-reduction on DVE: the ones
                        # matmul count drops 16 -> 4 per sqg
                        h = expool.tile([P, 512], BF16, tag="hs", bufs=4,
                                        name=f"hs{bi}{m}{sqg}{g}")
                        nc.vector.tensor_add(h, ex2[:, 0:512],
                                             ex2[:, 512:1024])
                        hsums[g] = h
                        if g % 2 == 1:
                            gg = g // 2
                            qs = expool.tile([P, 512], BF16, tag="qs", bufs=3,
                                             name=f"qs{bi}{m}{sqg}{gg}")
                            nc.vector.tensor_add(qs, hsums[g - 1], hsums[g])
                            hsums[g] = qs  # keep slot alive via dict
                            nc.tensor.matmul(
                                sum_ps, ones_sb, qs,
                                start=gg == 0, stop=gg == 3,
                            )
                        if micro:
                            for _ in range(micro_rate):
                                if micro:
                                    micro.pop(0)()
                    rec = recpool.tile([P, 512], F32, tag="rec", bufs=2,
                                       name=f"rec{bi}{m}{sqg}")
                    nc.vector.reciprocal_approx_fast(rec, sum_ps)
                    nc.vector.tensor_mul(outn[bi][:, m, qsl], out_ps, rec)
                    if fillers:
                        fillers.pop(0)()
                    if sqg_hook is not None:
                        sqg_hook(sqg)

            def emit_c_sqt(bi, sqt, po_pool, ob_pool):
                """o_proj for one 128-row seq block: 4 single-bank psum
                steps, drains alternating between ACT and DVE."""
                ob = ob_pool.tile([P, HIDDEN], BF16, tag="ob", bufs=3,
                                  name=f"ob{bi}{sqt}")
                ssl = slice(sqt * P, (sqt + 1) * P)
                for hc in range(4):
                    po = po_pool.tile([P, 512], F32, tag="po", bufs=2,
                                      name=f"po{bi}{sqt}{hc}")
                    col = hc * 512
                    for dc in range(NH_LOC):
                        nc.tensor.matmul(
                            po,
                            outn[bi][:, dc, ssl],
                            wo_sb[:, dc, col : col + 512],
                            start=dc == 0, stop=dc == NH_LOC - 1,
                        )
                    osl = slice(col, col + 512)
                    if hc % 2 == 0:
                        nc.scalar.copy(ob[:, osl], po)
                    else:
                        nc.vector.tensor_copy(ob[:, osl], po)
                    if hc % 2 == 1:
                        h2 = slice((hc - 1) * 512, (hc + 1) * 512)
                        nc.sync.dma_start(
                            out=out_d[
                                bi * S + sqt * P : bi * S + (sqt + 1) * P, h2
                            ],
                            in_=ob[:, h2],
                        )

            def emit_rot_vt_window(bi, w, pr_pool, tt_pool):
                """RoPE + vT for one 512-col window (window == rope chunk
                == vT group); emitted one window behind the projections so
                the PSUM drain is already done."""
                emit_rot_chunk(bi, kv_st[bi][:, 0, :], 0, w, pr_pool, tt_pool)
                emit_rot_chunk(bi, q_st[bi][:, 0, :], 1, w, pr_pool, tt_pool)
                emit_rot_chunk(bi, q_st[bi][:, 1, :], 2, w, pr_pool, tt_pool)
                emit_vt_group(bi, w, pr_pool)

            # ------------- the program -------------
            # A(b0): projection passes with window-local rope/vT trailing
            # one window behind (psA 4 banks + pr 2 + pv 2 = 8)
            psA_cm = tc.tile_pool(name="psA", bufs=2, space="PSUM")
            psA = psA_cm.__enter__()
            rv_cm = tc.tile_pool(name="rv0", bufs=2, space="PSUM")
            rv = rv_cm.__enter__()
            for w in range(4):
                emit_proj_pass(psA, 0, w, "kv")
                emit_proj_pass(psA, 0, w, "q")
                if w >= 1:
                    emit_rot_vt_window(0, w - 1, rv, stage)
            emit_rot_vt_window(0, 3, rv, stage)
            rv_cm.__exit__(None, None, None)
            psA_cm.__exit__(None, None, None)

            # B(b0,*): proj(b1) passes as fillers; h1's boundaries also
            # carry the k/q0 rope of b1 through spare psc slots
            fill_b1 = []
            psF_cm = tc.tile_pool(name="psF", bufs=1, space="PSUM")
            psF = psF_cm.__enter__()
            for w in range(4):
                for which in ("kv", "q"):
                    fill_b1.append(
                        lambda w=w, wh=which: emit_proj_pass(psF, 1, w, wh)
                    )

            def rope_pair_hook(psc_pool, jobs):
                """Two in-place rope chunks through one spare sc slot."""
                def hook(sqg, jobs=jobs):
                    pair = jobs.pop(0) if jobs else None
                    if pair is None:
                        return
                    sct = psc_pool.tile([P, 1024], F32, tag="sc",
                                        name=f"rp{pair[0][1]}{pair[0][2]}")
                    for half, (src_ap, ji, ch) in enumerate(pair):
                        sl = slice(ch * 512, (ch + 1) * 512)
                        pr = sct[:, half * 512 : (half + 1) * 512]
                        nc.tensor.matmul(pr, rt_sb, src_ap[:, sl],
                                         start=True, stop=True)
                        t_t = stage.tile([P, 512], BF16, tag="tt", bufs=4,
                                         name=f"tt1{ji}{ch}")
                        nc.vector.tensor_mul(t_t, pr, sin_sb[:, sl])
                        x_t = stage.tile([P, 512], BF16, tag="xt2", bufs=4,
                                         name=f"xt21{ji}{ch}")
                        nc.vector.tensor_mul(x_t, src_ap[:, sl],
                                             cos_sb[:, sl])
                        nc.vector.tensor_add(src_ap[:, sl], x_t, t_t)
                return hook

            k1_ap = kv_st[1][:, 0, :]
            q10_ap = q_st[1][:, 0, :]
            # boundary i's chunks only touch windows whose projection
            # filler has already been emitted (h0: w0/w1, h1 bN: see map)
            rope_b1_pairs = [
                [(k1_ap, 0, 0), (k1_ap, 0, 1)],
                [(k1_ap, 0, 2), (q10_ap, 1, 0)],
                [(k1_ap, 0, 3), (q10_ap, 1, 1)],
                [(q10_ap, 1, 2), (q10_ap, 1, 3)],
            ]

            for m in range(NH_LOC):
                pools_cm = [
                    tc.tile_pool(name=f"psc0{m}", bufs=2, space="PSUM"),
                    tc.tile_pool(name=f"pout0{m}", bufs=1, space="PSUM"),
                    tc.tile_pool(name=f"psum0{m}", bufs=1, space="PSUM"),
                    tc.tile_pool(name=f"ex0{m}", bufs=3),
                    tc.tile_pool(name=f"rec0{m}", bufs=2),
                ]
                pools = [p.__enter__() for p in pools_cm]
                hook = (rope_pair_hook(pools[0], rope_b1_pairs)
                        if m == 1 else None)
                emit_b_unit(0, m, pools, fillers=fill_b1, sqg_hook=hook)
                for p in reversed(pools_cm):
                    p.__exit__(None, None, None)
            psF_cm.__exit__(None, None, None)

            # vT(b1) + q1 rope of b1 (k/q0 were roped inside B(b0,h1))
            rv1_cm = tc.tile_pool(name="rv1", bufs=2, space="PSUM")
            rv1 = rv1_cm.__enter__()
            for g4 in range(4):
                emit_vt_group(1, g4, rv1)
            for ch in range(4):
                emit_rot_chunk(1, q_st[1][:, 1, :], 2, ch, rv1, stage)
            rv1_cm.__exit__(None, None, None)

            # B(b1,*): h0 carries C(b0) 2-sqt fillers (2 per boundary);
            # h1 carries window-gated C(b1) fillers; short dense tail
            poF_cm = tc.tile_pool(name="poF", bufs=2, space="PSUM")
            poF = poF_cm.__enter__()
            obF_cm = tc.tile_pool(name="obF", bufs=3)
            obF = obF_cm.__enter__()

            fill_c0 = []
            for quad in range(4):
                def filler(quad=quad):
                    for k in range(4):
                        emit_c_sqt(0, 4 * quad + k, poF, obF)
                fill_c0.append(filler)

            c1_next = [0]

            def c1_hook(sqg):
                # outn[1] window sqg complete -> rows 4*sqg..4*sqg+3 legal
                hi = min(4 * (sqg + 1), S // P)
                n = min(4, hi - c1_next[0])
                for _ in range(n):
                    emit_c_sqt(1, c1_next[0], poF, obF)
                    c1_next[0] += 1

            for m in range(NH_LOC):
                pools_cm = [
                    tc.tile_pool(name=f"psc1{m}", bufs=2, space="PSUM"),
                    tc.tile_pool(name=f"pout1{m}", bufs=1, space="PSUM"),
                    tc.tile_pool(name=f"psum1{m}", bufs=1, space="PSUM"),
                    tc.tile_pool(name=f"ex1{m}", bufs=3),
                    tc.tile_pool(name=f"rec1{m}", bufs=2),
                ]
                pools = [p.__enter__() for p in pools_cm]
                if m == 0:
                    emit_b_unit(1, m, pools, fillers=fill_c0)
                else:
                    emit_b_unit(1, m, pools, sqg_hook=c1_hook)
                for p in reversed(pools_cm):
                    p.__exit__(None, None, None)

            # dense tail: remaining C(b1) rows
            while c1_next[0] < S // P:
                emit_c_sqt(1, c1_next[0], poF, obF)
                c1_next[0] += 1
            obF_cm.__exit__(None, None, None)
            poF_cm.__exit__(None, None, None)
    nc.compile()
    return nc


_BUILD_CACHE = {}
LAST_RESULT = None


def _get_nc(add_mask):
    if add_mask not in _BUILD_CACHE:
        _BUILD_CACHE[add_mask] = _build(add_mask)
    return _BUILD_CACHE[add_mask]


def kernel(hidden_states, attention_mask, Wq, Wk, Wv, Wo):
    hidden_states = np.asarray(hidden_states, dtype=np.float32)
    attention_mask = np.asarray(attention_mask, dtype=np.float32)
    Wq = np.asarray(Wq, dtype=np.float32)
    Wk = np.asarray(Wk, dtype=np.float32)
    Wv = np.asarray(Wv, dtype=np.float32)
    Wo = np.asarray(Wo, dtype=np.float32)

    b, s, hidden = hidden_states.shape
    assert (b, s, hidden) == (B, S, HIDDEN)

    add_mask = bool(np.any(attention_mask))
    nc = _get_nc(add_mask)

    bf16 = ml_dtypes.bfloat16

    # X^T packed [p, w, c, s512]: hidden = c*128+p, seq-global = w*512+s
    xt = hidden_states.reshape(b * s, hidden).T  # [2048, 4096]
    xtb = np.ascontiguousarray(
        xt.reshape(KH, P, NW, 512).transpose(1, 2, 0, 3)
    ).astype(bf16)

    cos_t, sin_t = _rope_tables(s, HEAD_DIM, ROPE_THETA)
    cosb = cos_t.astype(bf16)
    rt = _rot_matrix_t(P).astype(bf16)
    ident = np.eye(P, dtype=np.float32).astype(bf16)
    onesb = np.ones((P, P), dtype=np.float32).astype(bf16)

    in_maps = []
    for c in range(N_CORES):
        kv = c // 2
        wq_c = Wq[:, c * NH_LOC * HEAD_DIM : (c + 1) * NH_LOC * HEAD_DIM]
        wk_c = Wk[:, kv * HEAD_DIM : (kv + 1) * HEAD_DIM]
        wv_c = Wv[:, kv * HEAD_DIM : (kv + 1) * HEAD_DIM]
        wo_c = Wo[c * NH_LOC * HEAD_DIM : (c + 1) * NH_LOC * HEAD_DIM, :]
        im = {
            "xtb": xtb,
            "wqb": np.ascontiguousarray(
                wq_c.reshape(KH, P, NH_LOC * P).transpose(1, 0, 2)
            ).astype(bf16),
            "wkb": np.ascontiguousarray(
                wk_c.reshape(KH, P, P).transpose(1, 0, 2)
            ).astype(bf16),
            "wvb": np.ascontiguousarray(
                wv_c.reshape(KH, P, P).transpose(1, 0, 2)
            ).astype(bf16),
            "wob": np.ascontiguousarray(
                wo_c.reshape(NH_LOC, P, HIDDEN).transpose(1, 0, 2)
            ).astype(bf16),
            "onesb": onesb,
            "cosb": cosb,
            "sinf": sin_t,
            "rt": rt,
            "ident": ident,
        }
        if add_mask:
            im["mask_t"] = np.ascontiguousarray(attention_mask[0, 0].T)
        in_maps.append(im)

    res = run_bass_kernel_spmd(nc, in_maps, core_ids=list(range(N_CORES)))
    global LAST_RESULT
    LAST_RESULT = res
    out = np.zeros((b * s, hidden), dtype=np.float32)
    for r in res.results:
        out += np.asarray(r["out"], dtype=np.float32)
    return out.reshape(b, s, hidden)
